# revision 11
# baseline (speedup 1.0000x reference)
"""Bass/Trainium2 kernel for nn_DenoisingTransformerLayer (GNN message passing).

Distribution: edges partitioned by destination node across 8 NeuronCores.
Each core owns a contiguous range of dst nodes; the segment softmax and
scatter-add stay core-local (no collectives). Host pre-gathers node features
per edge (transposed layouts for matmul), pads each 128-dst-node group to a
fixed tile count, and un-permutes outputs.

Math notes:
 - scores are clamped to [-5, 5], so softmax max-subtraction is unnecessary
   (exp in [6.7e-3, 148]); the reference's +1e-16 on the denominator is a
   no-op in fp32 because denom >= deg*e^-10 and typically >= 1.
 - alpha = ex/denom[dst] is factored: wV[n] = (1/denom[n]) * seg_sum(ex*(V+score)),
   so no per-edge division/gather-back is needed.
 - seg_sum is a one-hot matmul: onehot[e, n_local] built on-device via
   iota/is_equal from dst-local ids; PSUM accumulates over the group's tiles.
"""

import sys
from contextlib import ExitStack

import numpy as np

sys.path.insert(0, "/opt/trn_rl_repo")

import concourse.bass as bass  # noqa: E402
import concourse.mybir as mybir  # noqa: E402
import concourse.tile as tile  # noqa: E402
from concourse import bacc  # noqa: E402
from concourse.masks import make_identity  # noqa: E402

P = 128
H = 8
D = 16
HD = H * D  # 128
CLAMP = 5.0
NCORES = 8
NW = 8 + HD  # 136: [ex | W] per tile in segsum rhs
SQRT_D = 4.0  # sqrt(16)


def _quads(T):
    out = []
    t = 0
    while t < T:
        nq = min(4, T - t)
        out.append((t, nq))
        t += nq
    return out


def build_nc(G, T, bias_q=True, bias_e=True):
    """One SPMD NeuronCore program: G groups of 128 dst nodes, T 128-edge
    tiles per group (padded).

    PSUM accumulation discipline: start=True marks the whole 2KB bank region
    pending-zero, so each psum bank gets exactly one start=True opener per
    quad; later writes use start=False (fresh-write on pending bytes,
    accumulate on already-written bytes)."""
    S = G * T * P
    f32 = mybir.dt.float32
    nc = bacc.Bacc(None, target_bir_lowering=False, debug=False)

    eaT = nc.dram_tensor("eaT", [P, S], f32, kind="ExternalInput")
    xsT = nc.dram_tensor("xsT", [P, S], f32, kind="ExternalInput")
    xdT = nc.dram_tensor("xdT", [P, S], f32, kind="ExternalInput")
    dstl = nc.dram_tensor("dstl", [P, G * T], f32, kind="ExternalInput")
    wWQ = nc.dram_tensor("WQ", [P, HD], f32, kind="ExternalInput")
    wWK = nc.dram_tensor("WK", [P, HD], f32, kind="ExternalInput")
    wWV = nc.dram_tensor("WV", [P, HD], f32, kind="ExternalInput")
    wWE = nc.dram_tensor("WE1", [P, HD], f32, kind="ExternalInput")
    bQr = nc.dram_tensor("bQ", [1, HD], f32, kind="ExternalInput")
    bEr = nc.dram_tensor("bE1", [1, HD], f32, kind="ExternalInput")
    wE_o = nc.dram_tensor("wE_o", [P, S], f32, kind="ExternalOutput")
    wV_o = nc.dram_tensor("wV_o", [G * P, HD], f32, kind="ExternalOutput")

    mult = mybir.AluOpType.mult
    add = mybir.AluOpType.add
    is_eq = mybir.AluOpType.is_equal

    with tile.TileContext(nc) as tc, ExitStack() as ctx:
        const = ctx.enter_context(tc.tile_pool(name="const", bufs=1))
        io = ctx.enter_context(tc.tile_pool(name="io", bufs=2))
        work = ctx.enter_context(tc.tile_pool(name="work", bufs=2))
        ps = ctx.enter_context(tc.tile_pool(name="ps", bufs=1, space="PSUM"))
        ps_seg = ctx.enter_context(tc.tile_pool(name="ps_seg", bufs=2, space="PSUM"))

        # ---- constants ----
        iota_f = const.tile([P, P], f32)
        nc.gpsimd.iota(iota_f[:], pattern=[[1, P]], base=0, channel_multiplier=0,
                       allow_small_or_imprecise_dtypes=True)
        ident = const.tile([P, P], f32)
        make_identity(nc, ident[:])
        ones1 = const.tile([1, P], f32)
        nc.gpsimd.memset(ones1[:], 1.0)

        wq = const.tile([P, HD], f32)
        wk = const.tile([P, HD], f32)
        wv = const.tile([P, HD], f32)
        we = const.tile([P, HD], f32)
        bq = const.tile([1, HD], f32)
        be = const.tile([1, HD], f32)
        nc.sync.dma_start(out=wq[:], in_=wWQ[:])
        nc.sync.dma_start(out=wk[:], in_=wWK[:])
        nc.sync.dma_start(out=wv[:], in_=wWV[:])
        nc.sync.dma_start(out=we[:], in_=wWE[:])
        nc.sync.dma_start(out=bq[:], in_=bQr[:])
        nc.sync.dma_start(out=be[:], in_=bEr[:])
        dst_sb = const.tile([P, G * T], f32)
        nc.sync.dma_start(out=dst_sb[:], in_=dstl[:])

        for g in range(G):
            c0 = g * T * P
            ea_g = io.tile([P, T * P], f32, tag="ea")
            xs_g = io.tile([P, T * P], f32, tag="xs")
            xd_g = io.tile([P, T * P], f32, tag="xd")
            nc.sync.dma_start(out=ea_g[:], in_=eaT[:, c0 : c0 + T * P])
            nc.sync.dma_start(out=xs_g[:], in_=xsT[:, c0 : c0 + T * P])
            nc.sync.dma_start(out=xd_g[:], in_=xdT[:, c0 : c0 + T * P])
            wE_st = io.tile([P, T * P], f32, tag="wEst")
            segp = ps_seg.tile([P, NW], f32)

            for q0, nq in _quads(T):
                w4 = nq * P
                Qp = ps.tile([P, 512], f32, tag="Qp")
                Kp = ps.tile([P, 512], f32, tag="Kp")
                Vp = ps.tile([P, 512], f32, tag="Vp")
                Ep = ps.tile([P, 512], f32, tag="Ep")

                # optional bias matmuls open the Q/E banks (ones stationary)
                if bias_q or bias_e:
                    for j in range(nq):
                        jsl = slice(j * P, (j + 1) * P)
                        if bias_q:
                            nc.tensor.matmul(out=Qp[:, jsl], lhsT=ones1[:],
                                             rhs=bq[:], start=(j == 0), stop=False)
                        if bias_e:
                            nc.tensor.matmul(out=Ep[:, jsl], lhsT=ones1[:],
                                             rhs=be[:], start=(j == 0), stop=False)
                for j in range(nq):
                    t = q0 + j
                    jsl = slice(j * P, (j + 1) * P)
                    tsl = slice(t * P, (t + 1) * P)
                    last = j == nq - 1
                    nc.tensor.matmul(out=Qp[:, jsl], lhsT=xd_g[:, tsl], rhs=wq[:],
                                     start=(j == 0 and not bias_q), stop=last)
                    nc.tensor.matmul(out=Kp[:, jsl], lhsT=xs_g[:, tsl], rhs=wk[:],
                                     start=(j == 0), stop=last)
                    nc.tensor.matmul(out=Vp[:, jsl], lhsT=xs_g[:, tsl], rhs=wv[:],
                                     start=(j == 0), stop=False)
                    nc.tensor.matmul(out=Ep[:, jsl], lhsT=ea_g[:, tsl], rhs=we[:],
                                     start=(j == 0 and not bias_e), stop=last)

                # Q psum -> sbuf (frees a PSUM operand for the DVE multiplies)
                qsb = work.tile([P, 512], f32, tag="qsb")
                nc.scalar.copy(qsb[:, :w4], Qp[:, :w4])

                # score_e = K*Q*E  (score written straight into the wE staging tile)
                m1 = work.tile([P, 512], f32, tag="m1")
                nc.vector.tensor_tensor(out=m1[:, :w4], in0=Kp[:, :w4],
                                        in1=qsb[:, :w4], op=mult)
                sc = wE_st[:, q0 * P : q0 * P + w4]
                nc.vector.tensor_tensor(out=sc, in0=m1[:, :w4], in1=Ep[:, :w4], op=mult)

                # per-head scores: reduce D, clip (pre-scale by 4*CLAMP), exp(x/4)
                s8 = work.tile([P, 4 * H], f32, tag="s8")
                nc.vector.tensor_reduce(
                    out=s8[:, : nq * H],
                    in_=sc.rearrange("p (t h d) -> p (t h) d", h=H, d=D),
                    axis=mybir.AxisListType.X, op=add)
                nc.vector.tensor_scalar(
                    s8[:, : nq * H], s8[:, : nq * H],
                    SQRT_D * CLAMP, -SQRT_D * CLAMP,
                    op0=mybir.AluOpType.min, op1=mybir.AluOpType.max)
                rhsb = work.tile([P, 4 * NW], f32, tag="rhsb")
                rview = rhsb[:, : nq * NW].rearrange("p (t w) -> p t w", w=NW)
                nc.scalar.activation(
                    rview[:, :, 0:8],
                    s8[:, : nq * H].rearrange("p (t h) -> p t h", h=H),
                    mybir.ActivationFunctionType.Exp, scale=1.0 / SQRT_D)

                # V + score via identity matmul into Vp
                for j in range(nq):
                    t = q0 + j
                    nc.tensor.matmul(out=Vp[:, j * P : (j + 1) * P], lhsT=ident[:],
                                     rhs=wE_st[:, t * P : (t + 1) * P],
                                     start=False, stop=(j == nq - 1))

                # W = ex * (V + score) into rhs buffer W slots
                exb = rview[:, :, 0:8].to_broadcast([P, nq, H, D])
                nc.vector.tensor_tensor(
                    out=rview[:, :, 8:NW].rearrange("p t (h d) -> p t h d", d=D),
                    in0=Vp[:, :w4].rearrange("p (t h d) -> p t h d", h=H, d=D),
                    in1=exb, op=mult)

                # one-hot per tile (gpsimd) + segsum matmul accumulation
                for j in range(nq):
                    t = q0 + j
                    oh = work.tile([P, P], f32, tag=f"oh{j}")
                    nc.gpsimd.tensor_scalar(
                        oh[:], iota_f[:], dst_sb[:, g * T + t : g * T + t + 1], None,
                        op0=is_eq)
                    nc.tensor.matmul(out=segp[:], lhsT=oh[:],
                                     rhs=rhsb[:, j * NW : (j + 1) * NW],
                                     start=(t == 0), stop=(t == T - 1))

            # ---- group finalize: wV = seg_W / max(seg_ex, tiny) ----
            den = work.tile([P, H], f32, tag="den")
            nc.vector.tensor_scalar(den[:], segp[:, 0:8], 1e-20, None,
                                    op0=mybir.AluOpType.max)
            rec = work.tile([P, H], f32, tag="rec")
            nc.vector.reciprocal(rec[:], den[:])
            wvt = work.tile([P, HD], f32, tag="wvt")
            nc.vector.tensor_tensor(
                out=wvt[:].rearrange("p (h d) -> p h d", d=D),
                in0=segp[:, 8:NW].rearrange("p (h d) -> p h d", d=D),
                in1=rec[:].to_broadcast([P, H, D]), op=mult)
            nc.sync.dma_start(out=wV_o[g * P : (g + 1) * P, :], in_=wvt[:])
            nc.sync.dma_start(out=wE_o[:, c0 : c0 + T * P], in_=wE_st[:])

    nc.compile()
    return nc


def _prep(x, ea, src, dst, n_cores=NCORES):
    """Host-side sharding. Returns (G, T, in_maps, recover) where recover
    maps per-core outputs back to full (wV, wE)."""
    N = x.shape[0]
    Ne = ea.shape[0]
    NPC = -(-N // n_cores)  # ceil
    G = -(-NPC // P)

    core = dst // NPC
    loc = dst - core * NPC
    grp = loc // P
    lig = (loc % P).astype(np.float32)

    key = core * G + grp
    order = np.argsort(key, kind="stable")
    counts = np.bincount(key, minlength=n_cores * G)
    T = max(1, int(-(-counts.max() // P)))
    S = G * T * P

    starts = np.zeros(n_cores * G, np.int64)
    starts[1:] = np.cumsum(counts)[:-1]
    pos = np.empty(Ne, np.int64)
    pos[order] = np.arange(Ne) - starts[key[order]]
    eslot = grp * (T * P) + pos  # slot within the core's [0, S) range

    in_maps = []
    recover_info = []
    for c in range(n_cores):
        m = core == c
        e_ids = np.nonzero(m)[0]
        sl = eslot[e_ids]

        def packT(rows):
            full = np.zeros((S, P), np.float32)
            full[sl] = rows
            return np.ascontiguousarray(full.T)

        ea_c = packT(ea[e_ids])
        xs_c = packT(x[src[e_ids]])
        xd_c = packT(x[dst[e_ids]])
        dst_c = np.full((P, G * T), -1.0, np.float32)
        dst_c[sl % P, sl // P] = lig[e_ids]
        in_maps.append({"eaT": ea_c, "xsT": xs_c, "xdT": xd_c, "dstl": dst_c})
        recover_info.append((e_ids, sl))

    def recover(outs):
        wV = np.zeros((N, HD), np.float32)
        wE = np.zeros((Ne, HD), np.float32)
        for c in range(n_cores):
            e_ids, sl = recover_info[c]
            lo = c * NPC
            hi = min(N, lo + NPC)
            wV[lo:hi] = outs[c]["wV_o"][: hi - lo]
            wE[e_ids] = outs[c]["wE_o"].reshape(P, G * T, P)[sl % P, sl // P]
        return wV, wE

    return G, T, in_maps, recover


def kernel(x, edge_attr, edge_index, WQ, bQ, WK, WV, WE1, bE1):
    from concourse.bass_utils import run_bass_kernel_spmd

    x = np.asarray(x, np.float32)
    ea = np.asarray(edge_attr, np.float32)
    ei = np.asarray(edge_index)
    src = ei[0].astype(np.int64)
    dst = ei[1].astype(np.int64)

    G, T, in_maps, recover = _prep(x, ea, src, dst)

    weights = {
        "WQ": np.ascontiguousarray(np.asarray(WQ, np.float32)),
        "WK": np.ascontiguousarray(np.asarray(WK, np.float32)),
        "WV": np.ascontiguousarray(np.asarray(WV, np.float32)),
        "WE1": np.ascontiguousarray(np.asarray(WE1, np.float32)),
        "bQ": np.asarray(bQ, np.float32).reshape(1, HD).copy(),
        "bE1": np.asarray(bE1, np.float32).reshape(1, HD).copy(),
    }
    for m in in_maps:
        m.update(weights)

    nc = build_nc(G, T,
                  bias_q=bool(np.any(weights["bQ"])),
                  bias_e=bool(np.any(weights["bE1"])))
    res = run_bass_kernel_spmd(nc, in_maps, list(range(NCORES)))
    global LAST_EXEC_NS
    LAST_EXEC_NS = getattr(res, "exec_time_ns", None)
    wV, wE = recover(res.results)
    return wV.reshape(x.shape[0], H, D), wE


LAST_EXEC_NS = None


# revision 13
# speedup vs baseline: 52.4440x; 52.4440x over previous
"""Bass/Trainium2 kernel for nn_DenoisingTransformerLayer (GNN message passing).

Distribution: edges partitioned by destination node across 8 NeuronCores.
Each core owns a contiguous range of dst nodes; the segment softmax and
scatter-add stay core-local (no collectives). Host pre-gathers node features
per edge (transposed layouts for matmul), pads each 128-dst-node group to a
fixed tile count, and un-permutes outputs.

Math notes:
 - scores are clamped to [-5, 5], so softmax max-subtraction is unnecessary
   (exp in [6.7e-3, 148]); the reference's +1e-16 on the denominator is a
   no-op in fp32 because denom >= deg*e^-10 and typically >= 1.
 - alpha = ex/denom[dst] is factored: wV[n] = (1/denom[n]) * seg_sum(ex*(V+score)),
   so no per-edge division/gather-back is needed.
 - seg_sum is a one-hot matmul: onehot[e, n_local] built on-device via
   iota/is_equal from dst-local ids; PSUM accumulates over the group's tiles.
"""

import sys
from contextlib import ExitStack

import numpy as np

sys.path.insert(0, "/opt/trn_rl_repo")

import concourse.bass as bass  # noqa: E402
import concourse.mybir as mybir  # noqa: E402
import concourse.tile as tile  # noqa: E402
from concourse import bacc  # noqa: E402
from concourse.masks import make_identity  # noqa: E402

P = 128
H = 8
D = 16
HD = H * D  # 128
CLAMP = 5.0
NCORES = 8
NW = 8 + HD  # 136: [ex | W] per tile in segsum rhs
SQRT_D = 4.0  # sqrt(16)


def _quads(T):
    out = []
    t = 0
    while t < T:
        nq = min(4, T - t)
        out.append((t, nq))
        t += nq
    return out


def build_nc(G, T, bias_q=True, bias_e=True, repeat=1):
    """One SPMD NeuronCore program: G groups of 128 dst nodes, T 128-edge
    tiles per group (padded).

    PSUM accumulation discipline: start=True marks the whole 2KB bank region
    pending-zero, so each psum bank gets exactly one start=True opener per
    quad; later writes use start=False (fresh-write on pending bytes,
    accumulate on already-written bytes)."""
    S = G * T * P
    f32 = mybir.dt.float32
    nc = bacc.Bacc(None, target_bir_lowering=False, debug=False)

    eaT = nc.dram_tensor("eaT", [P, S], f32, kind="ExternalInput")
    xsT = nc.dram_tensor("xsT", [P, S], f32, kind="ExternalInput")
    xdT = nc.dram_tensor("xdT", [P, S], f32, kind="ExternalInput")
    dstl = nc.dram_tensor("dstl", [P, G * T], f32, kind="ExternalInput")
    wWQ = nc.dram_tensor("WQ", [P, HD], f32, kind="ExternalInput")
    wWK = nc.dram_tensor("WK", [P, HD], f32, kind="ExternalInput")
    wWV = nc.dram_tensor("WV", [P, HD], f32, kind="ExternalInput")
    wWE = nc.dram_tensor("WE1", [P, HD], f32, kind="ExternalInput")
    bQr = nc.dram_tensor("bQ", [1, HD], f32, kind="ExternalInput")
    bEr = nc.dram_tensor("bE1", [1, HD], f32, kind="ExternalInput")
    wE_o = nc.dram_tensor("wE_o", [P, S], f32, kind="ExternalOutput")
    wV_o = nc.dram_tensor("wV_o", [G * P, HD], f32, kind="ExternalOutput")

    mult = mybir.AluOpType.mult
    add = mybir.AluOpType.add
    is_eq = mybir.AluOpType.is_equal

    with tile.TileContext(nc) as tc, ExitStack() as ctx:
        const = ctx.enter_context(tc.tile_pool(name="const", bufs=1))
        io = ctx.enter_context(tc.tile_pool(name="io", bufs=2))
        work = ctx.enter_context(tc.tile_pool(name="work", bufs=2))
        ps = ctx.enter_context(tc.tile_pool(name="ps", bufs=1, space="PSUM"))
        ps_seg = ctx.enter_context(tc.tile_pool(name="ps_seg", bufs=2, space="PSUM"))

        # ---- constants ----
        iota_f = const.tile([P, P], f32)
        nc.gpsimd.iota(iota_f[:], pattern=[[1, P]], base=0, channel_multiplier=0,
                       allow_small_or_imprecise_dtypes=True)
        ident = const.tile([P, P], f32)
        make_identity(nc, ident[:])
        ones1 = const.tile([1, P], f32)
        nc.gpsimd.memset(ones1[:], 1.0)

        wq = const.tile([P, HD], f32)
        wk = const.tile([P, HD], f32)
        wv = const.tile([P, HD], f32)
        we = const.tile([P, HD], f32)
        bq = const.tile([1, HD], f32)
        be = const.tile([1, HD], f32)
        nc.sync.dma_start(out=wq[:], in_=wWQ[:])
        nc.sync.dma_start(out=wk[:], in_=wWK[:])
        nc.sync.dma_start(out=wv[:], in_=wWV[:])
        nc.sync.dma_start(out=we[:], in_=wWE[:])
        nc.sync.dma_start(out=bq[:], in_=bQr[:])
        nc.sync.dma_start(out=be[:], in_=bEr[:])
        dst_sb = const.tile([P, G * T], f32)
        nc.sync.dma_start(out=dst_sb[:], in_=dstl[:])

        for g in [g for _ in range(repeat) for g in range(G)]:
            c0 = g * T * P
            ea_g = io.tile([P, T * P], f32, tag="ea")
            xs_g = io.tile([P, T * P], f32, tag="xs")
            xd_g = io.tile([P, T * P], f32, tag="xd")
            nc.sync.dma_start(out=ea_g[:], in_=eaT[:, c0 : c0 + T * P])
            nc.sync.dma_start(out=xs_g[:], in_=xsT[:, c0 : c0 + T * P])
            nc.sync.dma_start(out=xd_g[:], in_=xdT[:, c0 : c0 + T * P])
            wE_st = io.tile([P, T * P], f32, tag="wEst")
            segp = ps_seg.tile([P, NW], f32)

            for q0, nq in _quads(T):
                w4 = nq * P
                Qp = ps.tile([P, 512], f32, tag="Qp")
                Kp = ps.tile([P, 512], f32, tag="Kp")
                Vp = ps.tile([P, 512], f32, tag="Vp")
                Ep = ps.tile([P, 512], f32, tag="Ep")

                # optional bias matmuls open the Q/E banks (ones stationary)
                if bias_q or bias_e:
                    for j in range(nq):
                        jsl = slice(j * P, (j + 1) * P)
                        if bias_q:
                            nc.tensor.matmul(out=Qp[:, jsl], lhsT=ones1[:],
                                             rhs=bq[:], start=(j == 0), stop=False)
                        if bias_e:
                            nc.tensor.matmul(out=Ep[:, jsl], lhsT=ones1[:],
                                             rhs=be[:], start=(j == 0), stop=False)
                for j in range(nq):
                    t = q0 + j
                    jsl = slice(j * P, (j + 1) * P)
                    tsl = slice(t * P, (t + 1) * P)
                    last = j == nq - 1
                    nc.tensor.matmul(out=Qp[:, jsl], lhsT=xd_g[:, tsl], rhs=wq[:],
                                     start=(j == 0 and not bias_q), stop=last)
                    nc.tensor.matmul(out=Kp[:, jsl], lhsT=xs_g[:, tsl], rhs=wk[:],
                                     start=(j == 0), stop=last)
                    nc.tensor.matmul(out=Vp[:, jsl], lhsT=xs_g[:, tsl], rhs=wv[:],
                                     start=(j == 0), stop=False)
                    nc.tensor.matmul(out=Ep[:, jsl], lhsT=ea_g[:, tsl], rhs=we[:],
                                     start=(j == 0 and not bias_e), stop=last)

                # Q psum -> sbuf (frees a PSUM operand for the DVE multiplies)
                qsb = work.tile([P, 512], f32, tag="qsb")
                nc.scalar.copy(qsb[:, :w4], Qp[:, :w4])

                # score_e = K*Q*E  (score written straight into the wE staging tile)
                m1 = work.tile([P, 512], f32, tag="m1")
                nc.vector.tensor_tensor(out=m1[:, :w4], in0=Kp[:, :w4],
                                        in1=qsb[:, :w4], op=mult)
                sc = wE_st[:, q0 * P : q0 * P + w4]
                nc.vector.tensor_tensor(out=sc, in0=m1[:, :w4], in1=Ep[:, :w4], op=mult)

                # per-head scores: reduce D, clip (pre-scale by 4*CLAMP), exp(x/4)
                s8 = work.tile([P, 4 * H], f32, tag="s8")
                nc.vector.tensor_reduce(
                    out=s8[:, : nq * H],
                    in_=sc.rearrange("p (t h d) -> p (t h) d", h=H, d=D),
                    axis=mybir.AxisListType.X, op=add)
                nc.vector.tensor_scalar(
                    s8[:, : nq * H], s8[:, : nq * H],
                    SQRT_D * CLAMP, -SQRT_D * CLAMP,
                    op0=mybir.AluOpType.min, op1=mybir.AluOpType.max)
                rhsb = work.tile([P, 4 * NW], f32, tag="rhsb")
                rview = rhsb[:, : nq * NW].rearrange("p (t w) -> p t w", w=NW)
                nc.scalar.activation(
                    rview[:, :, 0:8],
                    s8[:, : nq * H].rearrange("p (t h) -> p t h", h=H),
                    mybir.ActivationFunctionType.Exp, scale=1.0 / SQRT_D)

                # V + score via identity matmul into Vp
                for j in range(nq):
                    t = q0 + j
                    nc.tensor.matmul(out=Vp[:, j * P : (j + 1) * P], lhsT=ident[:],
                                     rhs=wE_st[:, t * P : (t + 1) * P],
                                     start=False, stop=(j == nq - 1))

                # W = ex * (V + score) into rhs buffer W slots
                exb = rview[:, :, 0:8].to_broadcast([P, nq, H, D])
                nc.vector.tensor_tensor(
                    out=rview[:, :, 8:NW].rearrange("p t (h d) -> p t h d", d=D),
                    in0=Vp[:, :w4].rearrange("p (t h d) -> p t h d", h=H, d=D),
                    in1=exb, op=mult)

                # one-hot per tile (gpsimd) + segsum matmul accumulation
                for j in range(nq):
                    t = q0 + j
                    oh = work.tile([P, P], f32, tag=f"oh{j}")
                    nc.gpsimd.tensor_scalar(
                        oh[:], iota_f[:], dst_sb[:, g * T + t : g * T + t + 1], None,
                        op0=is_eq)
                    nc.tensor.matmul(out=segp[:], lhsT=oh[:],
                                     rhs=rhsb[:, j * NW : (j + 1) * NW],
                                     start=(t == 0), stop=(t == T - 1))

            # ---- group finalize: wV = seg_W / max(seg_ex, tiny) ----
            den = work.tile([P, H], f32, tag="den")
            nc.vector.tensor_scalar(den[:], segp[:, 0:8], 1e-20, None,
                                    op0=mybir.AluOpType.max)
            rec = work.tile([P, H], f32, tag="rec")
            nc.vector.reciprocal(rec[:], den[:])
            wvt = work.tile([P, HD], f32, tag="wvt")
            nc.vector.tensor_tensor(
                out=wvt[:].rearrange("p (h d) -> p h d", d=D),
                in0=segp[:, 8:NW].rearrange("p (h d) -> p h d", d=D),
                in1=rec[:].to_broadcast([P, H, D]), op=mult)
            nc.sync.dma_start(out=wV_o[g * P : (g + 1) * P, :], in_=wvt[:])
            nc.sync.dma_start(out=wE_o[:, c0 : c0 + T * P], in_=wE_st[:])

    nc.compile()
    return nc


def _prep(x, ea, src, dst, n_cores=NCORES):
    """Host-side sharding. Returns (G, T, in_maps, recover) where recover
    maps per-core outputs back to full (wV, wE)."""
    N = x.shape[0]
    Ne = ea.shape[0]
    NPC = -(-N // n_cores)  # ceil
    G = -(-NPC // P)

    core = dst // NPC
    loc = dst - core * NPC
    grp = loc // P
    lig = (loc % P).astype(np.float32)

    key = core * G + grp
    order = np.argsort(key, kind="stable")
    counts = np.bincount(key, minlength=n_cores * G)
    T = max(1, int(-(-counts.max() // P)))
    S = G * T * P

    starts = np.zeros(n_cores * G, np.int64)
    starts[1:] = np.cumsum(counts)[:-1]
    pos = np.empty(Ne, np.int64)
    pos[order] = np.arange(Ne) - starts[key[order]]
    eslot = grp * (T * P) + pos  # slot within the core's [0, S) range

    in_maps = []
    recover_info = []
    for c in range(n_cores):
        m = core == c
        e_ids = np.nonzero(m)[0]
        sl = eslot[e_ids]

        def packT(rows):
            full = np.zeros((S, P), np.float32)
            full[sl] = rows
            return np.ascontiguousarray(full.T)

        ea_c = packT(ea[e_ids])
        xs_c = packT(x[src[e_ids]])
        xd_c = packT(x[dst[e_ids]])
        dst_c = np.full((P, G * T), -1.0, np.float32)
        dst_c[sl % P, sl // P] = lig[e_ids]
        in_maps.append({"eaT": ea_c, "xsT": xs_c, "xdT": xd_c, "dstl": dst_c})
        recover_info.append((e_ids, sl))

    def recover(outs):
        wV = np.zeros((N, HD), np.float32)
        wE = np.zeros((Ne, HD), np.float32)
        for c in range(n_cores):
            e_ids, sl = recover_info[c]
            lo = c * NPC
            hi = min(N, lo + NPC)
            wV[lo:hi] = outs[c]["wV_o"][: hi - lo]
            wE[e_ids] = outs[c]["wE_o"].reshape(P, G * T, P)[sl % P, sl // P]
        return wV, wE

    return G, T, in_maps, recover


def kernel(x, edge_attr, edge_index, WQ, bQ, WK, WV, WE1, bE1):
    from concourse.bass_utils import run_bass_kernel_spmd

    x = np.asarray(x, np.float32)
    ea = np.asarray(edge_attr, np.float32)
    ei = np.asarray(edge_index)
    src = ei[0].astype(np.int64)
    dst = ei[1].astype(np.int64)

    G, T, in_maps, recover = _prep(x, ea, src, dst)

    weights = {
        "WQ": np.ascontiguousarray(np.asarray(WQ, np.float32)),
        "WK": np.ascontiguousarray(np.asarray(WK, np.float32)),
        "WV": np.ascontiguousarray(np.asarray(WV, np.float32)),
        "WE1": np.ascontiguousarray(np.asarray(WE1, np.float32)),
        "bQ": np.asarray(bQ, np.float32).reshape(1, HD).copy(),
        "bE1": np.asarray(bE1, np.float32).reshape(1, HD).copy(),
    }
    for m in in_maps:
        m.update(weights)

    nc = build_nc(G, T,
                  bias_q=bool(np.any(weights["bQ"])),
                  bias_e=bool(np.any(weights["bE1"])))
    res = run_bass_kernel_spmd(nc, in_maps, list(range(NCORES)))
    global LAST_EXEC_NS
    LAST_EXEC_NS = getattr(res, "exec_time_ns", None)
    wV, wE = recover(res.results)
    return wV.reshape(x.shape[0], H, D), wE


LAST_EXEC_NS = None


# revision 22
# speedup vs baseline: 118.0372x; 2.2507x over previous
"""Bass/Trainium2 kernel for nn_DenoisingTransformerLayer (GNN message passing).

Distribution: edges partitioned by destination node across 8 NeuronCores.
Each core owns a contiguous range of dst nodes; the segment softmax and
scatter-add stay core-local (no collectives). Host pre-gathers node features
per edge (transposed layouts for matmul), pads each 128-dst-node group to a
fixed tile count, and un-permutes outputs.

Math notes:
 - scores are clamped to [-5, 5], so softmax max-subtraction is unnecessary
   (exp in [6.7e-3, 148]); the reference's +1e-16 on the denominator is a
   no-op in fp32 because denom >= deg*e^-10 and typically >= 1.
 - alpha = ex/denom[dst] is factored: wV[n] = (1/denom[n]) * seg_sum(ex*(V+score)),
   so no per-edge division/gather-back is needed.
 - seg_sum is a one-hot matmul: onehot[e, n_local] built on-device via
   iota/is_equal from dst-local ids; PSUM accumulates over the group's tiles.
"""

import sys
from contextlib import ExitStack

import numpy as np

sys.path.insert(0, "/opt/trn_rl_repo")

import concourse.bass as bass  # noqa: E402
import concourse.mybir as mybir  # noqa: E402
import concourse.tile as tile  # noqa: E402
from concourse import bacc  # noqa: E402
from concourse.masks import make_identity  # noqa: E402

P = 128
H = 8
D = 16
HD = H * D  # 128
CLAMP = 5.0
NCORES = 8
NW = 8 + HD  # 136: [ex | W] per tile in segsum rhs
SQRT_D = 4.0  # sqrt(16)


def _quads(T):
    out = []
    t = 0
    while t < T:
        nq = min(4, T - t)
        out.append((t, nq))
        t += nq
    return out


def build_nc(G, T, bias_q=True, bias_e=True, repeat=1, use_f32r=False):
    """One SPMD NeuronCore program: G groups of 128 dst nodes, T 128-edge
    tiles per group (padded).

    PSUM accumulation discipline: start=True marks the whole 2KB bank region
    pending-zero, so each psum bank gets exactly one start=True opener per
    quad; later writes use start=False (fresh-write on pending bytes,
    accumulate on already-written bytes)."""
    S = G * T * P
    f32 = mybir.dt.float32
    nc = bacc.Bacc(None, target_bir_lowering=False, debug=False)

    eaT = nc.dram_tensor("eaT", [P, S], f32, kind="ExternalInput")
    xsT = nc.dram_tensor("xsT", [P, S], f32, kind="ExternalInput")
    xdT = nc.dram_tensor("xdT", [P, S], f32, kind="ExternalInput")
    dstl = nc.dram_tensor("dstl", [P, G * T], f32, kind="ExternalInput")
    wWQ = nc.dram_tensor("WQ", [P, HD], f32, kind="ExternalInput")
    wWK = nc.dram_tensor("WK", [P, HD], f32, kind="ExternalInput")
    wWV = nc.dram_tensor("WV", [P, HD], f32, kind="ExternalInput")
    wWE = nc.dram_tensor("WE1", [P, HD], f32, kind="ExternalInput")
    bQr = nc.dram_tensor("bQ", [1, HD], f32, kind="ExternalInput")
    bEr = nc.dram_tensor("bE1", [1, HD], f32, kind="ExternalInput")
    wE_o = nc.dram_tensor("wE_o", [P, S], f32, kind="ExternalOutput")
    wV_o = nc.dram_tensor("wV_o", [G * P, HD], f32, kind="ExternalOutput")

    mult = mybir.AluOpType.mult
    add = mybir.AluOpType.add
    is_eq = mybir.AluOpType.is_equal

    # float32r single-pass PE mode (vs fp32's two half-rate passes) for the
    # wV-path matmuls only; needs moving dim >= 256 for full rate, so the
    # segsum rhs is padded 136 -> 256. wE path stays exact fp32.
    NWr = 256 if use_f32r else NW

    def r(ap):
        return ap.bitcast(mybir.dt.float32r) if use_f32r else ap

    with tile.TileContext(nc) as tc, ExitStack() as ctx:
        const = ctx.enter_context(tc.tile_pool(name="const", bufs=1))
        io = ctx.enter_context(tc.tile_pool(name="io", bufs=2))
        work = ctx.enter_context(tc.tile_pool(name="work", bufs=2))
        ps = ctx.enter_context(tc.tile_pool(name="ps", bufs=1, space="PSUM"))
        psb = ctx.enter_context(tc.tile_pool(name="psb", bufs=2, space="PSUM"))
        ps_seg = ctx.enter_context(tc.tile_pool(name="ps_seg", bufs=2, space="PSUM"))

        # ---- constants ----
        iota_f = const.tile([P, P], f32)
        nc.gpsimd.iota(iota_f[:], pattern=[[1, P]], base=0, channel_multiplier=0,
                       allow_small_or_imprecise_dtypes=True)
        ident = const.tile([P, P], f32)
        make_identity(nc, ident[:])
        ones1 = const.tile([1, P], f32)
        nc.gpsimd.memset(ones1[:], 1.0)

        wq = const.tile([P, HD], f32)
        wk = const.tile([P, HD], f32)
        wv = const.tile([P, HD], f32)
        we = const.tile([P, HD], f32)
        bq = const.tile([1, HD], f32)
        be = const.tile([1, HD], f32)
        nc.sync.dma_start(out=wq[:], in_=wWQ[:])
        nc.sync.dma_start(out=wk[:], in_=wWK[:])
        nc.sync.dma_start(out=wv[:], in_=wWV[:])
        nc.sync.dma_start(out=we[:], in_=wWE[:])
        nc.sync.dma_start(out=bq[:], in_=bQr[:])
        nc.sync.dma_start(out=be[:], in_=bEr[:])
        dst_sb = const.tile([P, G * T], f32)
        nc.sync.dma_start(out=dst_sb[:], in_=dstl[:])

        for g in [g for _ in range(repeat) for g in range(G)]:
            c0 = g * T * P
            ea_g = io.tile([P, T * P], f32, tag="ea")
            xs_g = io.tile([P, T * P], f32, tag="xs")
            xd_g = io.tile([P, T * P], f32, tag="xd")
            nc.sync.dma_start(out=ea_g[:], in_=eaT[:, c0 : c0 + T * P])
            nc.sync.dma_start(out=xs_g[:], in_=xsT[:, c0 : c0 + T * P])
            nc.sync.dma_start(out=xd_g[:], in_=xdT[:, c0 : c0 + T * P])
            wE_st = io.tile([P, T * P], f32, tag="wEst")
            segp = ps_seg.tile([P, NWr], f32)

            for q0, nq in _quads(T):
                w4 = nq * P
                Qp = ps.tile([P, 512], f32, tag="Qp")
                Kp = ps.tile([P, 512], f32, tag="Kp")
                Vp = psb.tile([P, 512], f32, tag="Vp")
                Ep = psb.tile([P, 512], f32, tag="Ep")

                # optional bias matmuls open the Q/E banks (ones stationary)
                if bias_q or bias_e:
                    for j in range(nq):
                        jsl = slice(j * P, (j + 1) * P)
                        if bias_q:
                            nc.tensor.matmul(out=Qp[:, jsl], lhsT=ones1[:],
                                             rhs=bq[:], start=(j == 0), stop=False)
                        if bias_e:
                            nc.tensor.matmul(out=Ep[:, jsl], lhsT=ones1[:],
                                             rhs=be[:], start=(j == 0), stop=False)
                for j in range(nq):
                    t = q0 + j
                    jsl = slice(j * P, (j + 1) * P)
                    tsl = slice(t * P, (t + 1) * P)
                    last = j == nq - 1
                    nc.tensor.matmul(out=Qp[:, jsl], lhsT=xd_g[:, tsl], rhs=wq[:],
                                     start=(j == 0 and not bias_q), stop=last)
                    nc.tensor.matmul(out=Kp[:, jsl], lhsT=xs_g[:, tsl], rhs=wk[:],
                                     start=(j == 0), stop=last)
                    nc.tensor.matmul(out=Vp[:, jsl], lhsT=xs_g[:, tsl], rhs=wv[:],
                                     start=(j == 0), stop=False)
                    nc.tensor.matmul(out=Ep[:, jsl], lhsT=ea_g[:, tsl], rhs=we[:],
                                     start=(j == 0 and not bias_e), stop=last)

                # Q psum -> sbuf (frees a PSUM operand for the DVE multiplies)
                qsb = work.tile([P, 512], f32, tag="qsb")
                nc.scalar.copy(qsb[:, :w4], Qp[:, :w4])

                # score_e = K*Q*E  (score written straight into the wE staging tile)
                m1 = work.tile([P, 512], f32, tag="m1")
                nc.vector.tensor_tensor(out=m1[:, :w4], in0=Kp[:, :w4],
                                        in1=qsb[:, :w4], op=mult)
                sc = wE_st[:, q0 * P : q0 * P + w4]
                nc.vector.tensor_tensor(out=sc, in0=m1[:, :w4], in1=Ep[:, :w4], op=mult)

                # per-head scores: reduce D, clip (pre-scale by 4*CLAMP), exp(x/4)
                s8 = work.tile([P, 4 * H], f32, tag="s8")
                nc.vector.tensor_reduce(
                    out=s8[:, : nq * H],
                    in_=sc.rearrange("p (t h d) -> p (t h) d", h=H, d=D),
                    axis=mybir.AxisListType.X, op=add)
                nc.vector.tensor_scalar(
                    s8[:, : nq * H], s8[:, : nq * H],
                    SQRT_D * CLAMP, -SQRT_D * CLAMP,
                    op0=mybir.AluOpType.min, op1=mybir.AluOpType.max)
                rhsb = work.tile([P, 4 * NWr], f32, tag="rhsb")
                rview = rhsb[:, : nq * NWr].rearrange("p (t w) -> p t w", w=NWr)
                if use_f32r:
                    # pad cols feed the (ignored) segp[136:256] region; memset
                    # keeps them finite for simulators
                    nc.gpsimd.memset(rview[:, :, NW:NWr], 0.0)
                nc.scalar.activation(
                    rview[:, :, 0:8],
                    s8[:, : nq * H].rearrange("p (t h) -> p t h", h=H),
                    mybir.ActivationFunctionType.Exp, scale=1.0 / SQRT_D)

                # V + score via one identity matmul (N = nq*128 <= 512) into Vp
                nc.tensor.matmul(out=Vp[:, :w4], lhsT=r(ident[:]),
                                 rhs=r(wE_st[:, q0 * P : q0 * P + w4]),
                                 start=False, stop=True)

                # W = ex * (V + score) into rhs buffer W slots
                exb = rview[:, :, 0:8].to_broadcast([P, nq, H, D])
                nc.vector.tensor_tensor(
                    out=rview[:, :, 8:NW].rearrange("p t (h d) -> p t h d", d=D),
                    in0=Vp[:, :w4].rearrange("p (t h d) -> p t h d", h=H, d=D),
                    in1=exb, op=mult)

                # one-hot per tile (gpsimd) + segsum matmul accumulation
                for j in range(nq):
                    t = q0 + j
                    oh = work.tile([P, P], f32, tag=f"oh{j}")
                    nc.gpsimd.tensor_scalar(
                        oh[:], iota_f[:], dst_sb[:, g * T + t : g * T + t + 1], None,
                        op0=is_eq)
                    nc.tensor.matmul(out=segp[:], lhsT=r(oh[:]),
                                     rhs=r(rhsb[:, j * NWr : (j + 1) * NWr]),
                                     start=(t == 0), stop=(t == T - 1))

            # ---- group finalize: wV = seg_W / max(seg_ex, tiny) ----
            den = work.tile([P, H], f32, tag="den")
            nc.vector.tensor_scalar(den[:], segp[:, 0:8], 1e-20, None,
                                    op0=mybir.AluOpType.max)
            rec = work.tile([P, H], f32, tag="rec")
            nc.vector.reciprocal(rec[:], den[:])
            wvt = work.tile([P, HD], f32, tag="wvt")
            nc.vector.tensor_tensor(
                out=wvt[:].rearrange("p (h d) -> p h d", d=D),
                in0=segp[:, 8:NW].rearrange("p (h d) -> p h d", d=D),
                in1=rec[:].to_broadcast([P, H, D]), op=mult)
            nc.sync.dma_start(out=wV_o[g * P : (g + 1) * P, :], in_=wvt[:])
            nc.sync.dma_start(out=wE_o[:, c0 : c0 + T * P], in_=wE_st[:])

    nc.compile()
    return nc


def _prep(x, ea, src, dst, n_cores=NCORES):
    """Host-side sharding. Returns (G, T, in_maps, recover) where recover
    maps per-core outputs back to full (wV, wE)."""
    N = x.shape[0]
    Ne = ea.shape[0]
    NPC = -(-N // n_cores)  # ceil
    G = -(-NPC // P)

    core = dst // NPC
    loc = dst - core * NPC
    grp = loc // P
    lig = (loc % P).astype(np.float32)

    key = core * G + grp
    order = np.argsort(key, kind="stable")
    counts = np.bincount(key, minlength=n_cores * G)
    T = max(1, int(-(-counts.max() // P)))
    S = G * T * P

    starts = np.zeros(n_cores * G, np.int64)
    starts[1:] = np.cumsum(counts)[:-1]
    pos = np.empty(Ne, np.int64)
    pos[order] = np.arange(Ne) - starts[key[order]]
    eslot = grp * (T * P) + pos  # slot within the core's [0, S) range

    in_maps = []
    recover_info = []
    for c in range(n_cores):
        m = core == c
        e_ids = np.nonzero(m)[0]
        sl = eslot[e_ids]

        def packT(rows):
            full = np.zeros((S, P), np.float32)
            full[sl] = rows
            return np.ascontiguousarray(full.T)

        ea_c = packT(ea[e_ids])
        xs_c = packT(x[src[e_ids]])
        xd_c = packT(x[dst[e_ids]])
        dst_c = np.full((P, G * T), -1.0, np.float32)
        dst_c[sl % P, sl // P] = lig[e_ids]
        in_maps.append({"eaT": ea_c, "xsT": xs_c, "xdT": xd_c, "dstl": dst_c})
        recover_info.append((e_ids, sl))

    def recover(outs):
        wV = np.zeros((N, HD), np.float32)
        wE = np.zeros((Ne, HD), np.float32)
        for c in range(n_cores):
            e_ids, sl = recover_info[c]
            lo = c * NPC
            hi = min(N, lo + NPC)
            wV[lo:hi] = outs[c]["wV_o"][: hi - lo]
            wE[e_ids] = outs[c]["wE_o"].reshape(P, G * T, P)[sl % P, sl // P]
        return wV, wE

    return G, T, in_maps, recover


def kernel(x, edge_attr, edge_index, WQ, bQ, WK, WV, WE1, bE1):
    from concourse.bass_utils import run_bass_kernel_spmd

    x = np.asarray(x, np.float32)
    ea = np.asarray(edge_attr, np.float32)
    ei = np.asarray(edge_index)
    src = ei[0].astype(np.int64)
    dst = ei[1].astype(np.int64)

    G, T, in_maps, recover = _prep(x, ea, src, dst)

    weights = {
        "WQ": np.ascontiguousarray(np.asarray(WQ, np.float32)),
        "WK": np.ascontiguousarray(np.asarray(WK, np.float32)),
        "WV": np.ascontiguousarray(np.asarray(WV, np.float32)),
        "WE1": np.ascontiguousarray(np.asarray(WE1, np.float32)),
        "bQ": np.asarray(bQ, np.float32).reshape(1, HD).copy(),
        "bE1": np.asarray(bE1, np.float32).reshape(1, HD).copy(),
    }
    for m in in_maps:
        m.update(weights)

    nc = build_nc(G, T,
                  bias_q=bool(np.any(weights["bQ"])),
                  bias_e=bool(np.any(weights["bE1"])))
    res = run_bass_kernel_spmd(nc, in_maps, list(range(NCORES)))
    global LAST_EXEC_NS
    LAST_EXEC_NS = getattr(res, "exec_time_ns", None)
    wV, wE = recover(res.results)
    return wV.reshape(x.shape[0], H, D), wE


LAST_EXEC_NS = None
